# revision 1
# baseline (speedup 1.0000x reference)
"""DPConv (kernel=8, ext=4, stride=4) on 8 TRN2 NeuronCores.

Math: with K = k + 2e = 16 and k = 8, every adaptive-pool bin is exactly
2 wide, so the whole DPConv collapses to a separable linear operator:

    out_img = L @ img @ L.T          (per (n, c) image, 128x128)

where L is a 128x128 stencil matrix: for output index w the contributing
windows are i in [max(0,ceil((w-7)/4)), min(30, floor(w/4))] (counted
twice when that range is a single i - the fold count normalization),
each contributing the clamped replicate-padded pair {2w-4i-4, 2w-4i-3}
with weight 1/4 (pool avg 1/2 x fold avg 1/2).

The kernel is pure DMA-bound (target_regime=memory): 2 MiB in + 2 MiB
out per core.  Both directions ride bf16 (the rel-err budget is 2e-2;
bf16 end-to-end lands at ~5e-3), which halves HBM traffic vs the fp32
baseline.  L is exact in bf16 (entries are n/16), so a single full-rate
bf16 matmul does the row pass with fp32 PSUM accumulation.

On-chip factorization per image tile [H=128 partitions, W=128 free]:
  rows:  T = (L/4) @ x on TensorE (PSUM f32).  The HOST permutes each
         image's columns to [evens | odds] so the pairsum reads two
         contiguous 64-col blocks.
  cols:  P[v] = T_e[v-2] + T_o[v-2] (pairsum: ACT evacuates the even
         block PSUM->SBUF, DVE adds the odd block straight from PSUM,
         bf16 out).  Fold out[4a+b] = P[2a+b] + P[2a+b+2] in bf16,
         split DVE (a=1..22) / GpSimd (a=23..30).  Edge cols ride ACT.

DMA schedule - the critical lesson from traces: a single HWDGE queue
drains strictly FIFO at ~250 GB/s, so stores queued behind loads
cannot start until the entire input stream has drained.  v7 therefore
splits directions across the two HWDGE rings:
  sync/SP ring:   lt, then loads [8, 8, 16, 16, 16] images
  ACT ring:       stores [16, 16, 16, 8, 8] images
Stores overlap the tail of the load stream and the two queues'
packets interleave in the shared SDMA engines.  Each store is issued
one compute-group late so its cross-engine wait (DVE/GpSimd fold
writes) is already satisfied and never head-of-line-blocks ACT's
evacuations.  All DMA access patterns are flattened to one long free
dim per partition (2-8 KiB descriptors).

Sharding: pure data parallel - core k takes batch element n = k.
Host staging: transpose to [H, C, W], permute W to [evens | odds],
cast bf16, flatten.  Output returns [H, C*W] bf16, upcast + transposed
on the host.
"""

import ml_dtypes
import numpy as np

import concourse.bacc as bacc
import concourse.mybir as mybir
import concourse.tile as tile
from concourse import bass_utils
from concourse.ap import AP

N_CORES = 8
C_PER_CORE = 64          # images per core (= C; one batch element per core)
G = 8                    # images per compute group (PSUM: [128,G,128] f32)
LOAD_CHUNKS = (8, 8, 16, 16, 16)     # images per input DMA (sync ring)
STORE_CHUNKS = (16, 16, 16, 8, 8)    # images per output DMA (ACT ring)
DVE_FOLD_A = 22          # fold a-slots on DVE (cols 4..4+4*A-1); rest GpSimd
N_GROUPS = C_PER_CORE // G
F32 = mybir.dt.float32
BF16 = mybir.dt.bfloat16
BF16_NP = ml_dtypes.bfloat16
assert sum(LOAD_CHUNKS) == C_PER_CORE and sum(STORE_CHUNKS) == C_PER_CORE

# host-side column permutation: [evens | odds]
_PERM = np.concatenate([np.arange(0, 128, 2), np.arange(1, 128, 2)])


def _build_lq() -> np.ndarray:
    """The 1-D DPConv operator with both 1/4 scalings folded in: L/4."""
    L = np.zeros((128, 128), np.float64)
    for w in range(128):
        i_lo = max(0, -((7 - w) // 4))      # ceil((w-7)/4)
        i_hi = min(30, w // 4)
        for i in (i_lo, i_hi):              # counted twice when equal
            L[w, min(127, max(0, 2 * w - 4 * i - 4))] += 0.25
            L[w, min(127, max(0, 2 * w - 4 * i - 3))] += 0.25
    return (L / 4.0).astype(np.float32)


_LQ_T = np.ascontiguousarray(_build_lq().T)          # lhsT layout [r, h]
_LQ_T_BF16 = _LQ_T.astype(BF16_NP)
assert np.all(_LQ_T_BF16.astype(np.float32) == _LQ_T)  # L exact in bf16


def _as_strided(base: AP, dims) -> AP:
    """Rebuild `base` (a sliced AP pointing at the wanted offset) with
    explicit [stride, size] free dims (overlapping reads allowed)."""
    return AP(base.tensor, base.offset, dims)


def _flat(ap: AP, n: int) -> AP:
    """Collapse a [128, c, w] access pattern into [128, c*w] so the DMA
    builder emits one long descriptor per partition instead of one per
    image row."""
    pdim = list(ap.ap[0])
    return AP(ap.tensor, ap.offset, [pdim, [1, n]])


def _chunk_starts(chunks):
    s, out = 0, []
    for c in chunks:
        out.append(s)
        s += c
    return out


def _dpconv_tile(tc, o_d, xb_d, lt_d):
    nc = tc.nc
    load_starts = _chunk_starts(LOAD_CHUNKS)
    store_starts = _chunk_starts(STORE_CHUNKS)
    with tc.tile_pool(name="const", bufs=1) as cp, \
         tc.tile_pool(name="in", bufs=1) as inp, \
         tc.tile_pool(name="io", bufs=1) as iop, \
         tc.tile_pool(name="mid", bufs=3) as mp, \
         tc.tile_pool(name="ps", bufs=4, space="PSUM") as pp:
        # lt first on the ring: 32 KiB, delays the first load by ~200ns
        # but lets the first matmul fire as soon as load0 lands.
        lt = cp.tile([128, 128], BF16)
        nc.sync.dma_start(out=lt[:], in_=lt_d)

        # all input DMAs issued up-front on the sync/SP HWDGE ring;
        # every chunk has its own buffer so nothing back-pressures.
        in_tiles = {}                         # image index -> (tile, offset)
        for i, (c0, cn) in enumerate(zip(load_starts, LOAD_CHUNKS)):
            ct = inp.tile([128, cn, 128], BF16, tag=f"in{i}", name=f"ct{i}")
            assert list(ct[:].ap[1])[0] == 128, "padded tile breaks _flat"
            nc.sync.dma_start(
                out=_flat(ct[:], cn * 128),
                in_=xb_d[:, c0 * 128:(c0 + cn) * 128])
            for c in range(c0, c0 + cn):
                in_tiles[c] = (ct, c - c0)

        out_tiles = {}
        store_ready_at = {}                  # group -> store chunk index
        for i, (c0, cn) in enumerate(zip(store_starts, STORE_CHUNKS)):
            ot = iop.tile([128, cn, 128], BF16, tag=f"out{i}", name=f"ot{i}")
            assert list(ot[:].ap[1])[0] == 128, "padded tile breaks _flat"
            for c in range(c0, c0 + cn):
                out_tiles[c] = (ot, c - c0)
            store_ready_at[(c0 + cn) // G - 1] = i

        def issue_store(i):
            c0, cn = store_starts[i], STORE_CHUNKS[i]
            ot, _ = out_tiles[c0]
            nc.scalar.dma_start(
                out=o_d[:, c0 * 128:(c0 + cn) * 128],
                in_=_flat(ot[:], cn * 128))

        pending_stores = []
        for g in range(N_GROUPS):
            ct, cofs = in_tiles[g * G]

            # rows: T = (L/4) @ x; one bf16 matmul per 512-col PSUM bank
            t1 = pp.tile([128, G, 128], F32, tag="t1")
            for h in range(G // 4):
                cs = slice(4 * h, 4 * (h + 1))
                nc.tensor.matmul(t1[:, cs, :], lt[:],
                                 ct[:, cofs + 4 * h:cofs + 4 * (h + 1)],
                                 start=True, stop=True)

            # cols step 1: pairsum P[v] = T_e[v-2] + T_o[v-2] - the host
            # permuted image cols to [evens | odds], so both reads are
            # contiguous 64-col blocks.  TensorTensor may read at most
            # ONE input from PSUM: ACT evacuates the even block, DVE
            # adds the odd block straight from PSUM, casting to bf16.
            pe_t = mp.tile([128, G, 64], F32, tag="pe")
            nc.scalar.copy(out=pe_t[:], in_=t1[:, :, 0:64])
            pt = mp.tile([128, G, 68], BF16, tag="P")
            gdim = list(pt[:].ap[1])            # [68-ish pitch, G]
            pdim0 = list(pt[:].ap[0])           # partition dim
            tdim = t1[:].ap
            nc.vector.tensor_add(
                out=pt[:, :, 2:66], in0=pe_t[:], in1=t1[:, :, 64:128])
            # P edge cols {0,1,66,67} = 2x permuted-T cols {0,0,127,127}
            # (= original image cols {0,127}): one ACT op - out strides
            # (66,1), in strides (127, 0-broadcast)
            nc.scalar.mul(
                _as_strided(pt[:, :, 0:1], [pdim0, gdim, [66, 2], [1, 2]]),
                _as_strided(t1[:, :, 0:1],
                            [list(tdim[0]), list(tdim[1]), [127, 2], [0, 2]]),
                2.0)

            # cols step 2: fold out[4a+b] = P[2a+b] + P[2a+b+2], with
            # overlapping as-strided bf16 reads and contiguous bf16
            # writes, split DVE (a=1..DVE_FOLD_A) / GpSimd (rest).
            # Out-edge cols {0..3,124..127} = 2x P{0..3,64..67} on ACT.
            ot, oofs = out_tiles[g * G]
            odim = ot[:, oofs:oofs + G, :].ap
            na = DVE_FOLD_A
            nb = 30 - na
            in0a = _as_strided(pt[:, :, 2:3], [pdim0, gdim, [2, na], [1, 4]])
            in1a = _as_strided(pt[:, :, 4:5], [pdim0, gdim, [2, na], [1, 4]])
            out_a = _as_strided(
                ot[:, oofs:oofs + G, 4:5],
                [list(odim[0]), list(odim[1]), [4, na], [1, 4]])
            nc.vector.tensor_add(out=out_a, in0=in0a, in1=in1a)
            pb = 2 + 2 * na
            in0b = _as_strided(pt[:, :, pb:pb + 1],
                               [pdim0, gdim, [2, nb], [1, 4]])
            in1b = _as_strided(pt[:, :, pb + 2:pb + 3],
                               [pdim0, gdim, [2, nb], [1, 4]])
            out_b = _as_strided(
                ot[:, oofs:oofs + G, 4 + 4 * na:5 + 4 * na],
                [list(odim[0]), list(odim[1]), [4, nb], [1, 4]])
            nc.gpsimd.tensor_add(out=out_b, in0=in0b, in1=in1b)
            edge_in = _as_strided(pt[:, :, 0:1], [pdim0, gdim, [64, 2], [1, 4]])
            edge_out = _as_strided(
                ot[:, oofs:oofs + G, 0:1],
                [list(odim[0]), list(odim[1]), [124, 2], [1, 4]])
            nc.scalar.mul(edge_out, edge_in, 2.0)

            # stores ride the ACT ring so they overlap the load stream
            # (single HWDGE queue = strict FIFO: stores queued behind
            # loads would wait for the whole input).  Each store is
            # issued one group late so its wait on the DVE/GpSimd fold
            # writes is already satisfied - no head-of-line block of
            # ACT's evacuations.
            for i in pending_stores:
                issue_store(i)
            pending_stores = []
            if g in store_ready_at:
                pending_stores.append(store_ready_at[g])
        for i in pending_stores:
            issue_store(i)


_CACHE = {}


def _get_nc():
    if "nc" not in _CACHE:
        nc = bacc.Bacc("TRN2", target_bir_lowering=False, debug=False)
        xb_d = nc.dram_tensor("xb", (128, C_PER_CORE * 128), BF16,
                              kind="ExternalInput").ap()
        lt_d = nc.dram_tensor("lt", (128, 128), BF16,
                              kind="ExternalInput").ap()
        o_d = nc.dram_tensor("o", (128, C_PER_CORE * 128), BF16,
                             kind="ExternalOutput").ap()
        with tile.TileContext(nc) as tc:
            _dpconv_tile(tc, o_d, xb_d, lt_d)
        nc.compile()
        _CACHE["nc"] = nc
    return _CACHE["nc"]


def _stage(xk: np.ndarray) -> np.ndarray:
    """[C,H,W] f32 -> [H, C*W] bf16, W permuted to [evens | odds] so
    the on-chip pairsum reads contiguous PSUM blocks; H-major so DMA
    reads are one long contiguous run per partition."""
    return np.ascontiguousarray(
        xk.transpose(1, 0, 2)[:, :, _PERM]).astype(BF16_NP).reshape(128, -1)


def run(x: np.ndarray, **spmd_kwargs) -> bass_utils.BassKernelResults:
    """Shard x (8,64,128,128) across 8 cores and run the Bass kernel."""
    nc = _get_nc()
    in_maps = [
        {"xb": _stage(x[k]), "lt": _LQ_T_BF16} for k in range(N_CORES)
    ]
    return bass_utils.run_bass_kernel_spmd(
        nc, in_maps, core_ids=list(range(N_CORES)), **spmd_kwargs)


def kernel(x) -> np.ndarray:
    x = np.asarray(x, dtype=np.float32)
    assert x.shape == (N_CORES, C_PER_CORE, 128, 128), x.shape
    res = run(x)
    return np.stack(
        [res.results[k]["o"].reshape(128, C_PER_CORE, 128)
         .astype(np.float32).transpose(1, 0, 2)
         for k in range(N_CORES)],
        axis=0)



# revision 3
# speedup vs baseline: 1.1022x; 1.1022x over previous
"""DPConv (kernel=8, ext=4, stride=4) on 8 TRN2 NeuronCores — v8.

Math: with K = k + 2e = 16 and k = 8, every adaptive-pool bin is exactly
2 wide, so DPConv collapses to out = L @ img @ L.T per (n, c) image,
where L (128x128, entries n/16) is the exact 1-D operator.  The column
pass further factors through natural pairs P[s] = x[2s] + x[2s+1]:

    out[:, 4a+b] = P[:, 2a+b-2] + P[:, 2a+b]      (a = 1..30, b = 0..3)
    out[:, {0,1}]   = T(x_col0)        out[:, {2,3}]     = 2 P[:, {0,1}]
    out[:, {126,127}] = T(x_col127)    out[:, {124,125}] = 2 P[:, {62,63}]

(P here carries the matmul's Lq = L/4 scaling, which absorbs the 1/4.)

v8 on-chip structure — the pairsum rides the matmul for free:
  The host stages each image as [evens(64) | odds(64)] and the kernel
  issues TWO accumulating matmuls per PSUM region (start/stop flags):
  PSUM <- Lq@evens + Lq@odds = P directly.  No vector-engine pairsum,
  no separate PSUM evacuation of T.  A small header block [lt | E]
  (E = [4*x_col0, 4*x_col127] per image) is loaded first; one extra
  matmul produces psumE = T(x_col0/127) for the outer edge columns.

  Per group of 8/16 images:
    ACT:  evacuate P PSUM->SBUF bf16 (one big copy)
    DVE:  fold_even + fold_odd  (interior cols 4..123) — both use
          4B-aligned 2-element runs so the 2x_1P bf16 mode can engage
    DVE:  e1 — copy psumE into out cols {0,1,126,127}
    GpS:  e3 — out cols {2,3,124,125} = 2 * P edge slots
    ACT:  store DMA issue (one group late, avoids head-of-line block)

DMA: loads on the sync/SP HWDGE ring, stores on the ACT ring; the two
rings share the 16 SDMA engines round-robin so load+store streams
overlap.  Chunks (8,8,16,16,16 images) = groups = store chunks; small
first chunks start the compute/store pipeline early.  All access
patterns flatten to one long free dim per partition (2-4 KiB descs).

Sharding: pure data parallel — core k takes batch element n = k.
Host staging: [H, C, W] transpose, W -> [evens|odds], edge cols
pre-scaled by 4, cast bf16.  Output returns [H, C*W] bf16.
"""

import ml_dtypes
import numpy as np

import concourse.bacc as bacc
import concourse.mybir as mybir
import concourse.tile as tile
from concourse import bass_utils
from concourse.ap import AP

N_CORES = 8
C_PER_CORE = 64          # images per core (= C; one batch element per core)
GROUPS = (8, 8, 16, 16, 16)   # images per load chunk == compute group == store
HDR = 256                # header cols: lt(128) + E(128)
F32 = mybir.dt.float32
BF16 = mybir.dt.bfloat16
BF16_NP = ml_dtypes.bfloat16
assert sum(GROUPS) == C_PER_CORE


def _build_lq() -> np.ndarray:
    """The 1-D DPConv operator with the column-pass 1/4 folded in: L/4."""
    L = np.zeros((128, 128), np.float64)
    for w in range(128):
        i_lo = max(0, -((7 - w) // 4))      # ceil((w-7)/4)
        i_hi = min(30, w // 4)
        for i in (i_lo, i_hi):              # counted twice when equal
            L[w, min(127, max(0, 2 * w - 4 * i - 4))] += 0.25
            L[w, min(127, max(0, 2 * w - 4 * i - 3))] += 0.25
    return (L / 4.0).astype(np.float32)


_LQ_T = np.ascontiguousarray(_build_lq().T)          # lhsT layout [r, h]
_LQ_T_BF16 = _LQ_T.astype(BF16_NP)
assert np.all(_LQ_T_BF16.astype(np.float32) == _LQ_T)  # L exact in bf16


def _as_strided(base: AP, dims) -> AP:
    """Rebuild `base` (a sliced AP pointing at the wanted offset) with
    explicit [stride, size] free dims (overlapping reads allowed)."""
    return AP(base.tensor, base.offset, dims)


def _flat(ap: AP, n: int) -> AP:
    """Collapse a [128, c, w] access pattern into [128, c*w] so the DMA
    builder emits one long descriptor per partition."""
    pdim = list(ap.ap[0])
    return AP(ap.tensor, ap.offset, [pdim, [1, n]])


def _starts(chunks):
    s, out = 0, []
    for c in chunks:
        out.append(s)
        s += c
    return out


def _dpconv_tile(tc, o_d, xb_d):
    nc = tc.nc
    g_starts = _starts(GROUPS)
    with tc.tile_pool(name="const", bufs=1) as cp, \
         tc.tile_pool(name="in", bufs=1) as inp, \
         tc.tile_pool(name="io", bufs=1) as iop, \
         tc.tile_pool(name="mid", bufs=3) as mp, \
         tc.tile_pool(name="ps", bufs=3, space="PSUM") as pp, \
         tc.tile_pool(name="psE", bufs=1, space="PSUM") as ppE:
        # header first on the ring: 64 KiB = lt + E-block; lets the E
        # matmul and first group's matmuls fire as soon as c0 lands.
        hdr = cp.tile([128, HDR], BF16)
        nc.sync.dma_start(out=hdr[:], in_=xb_d[:, 0:HDR])
        lt = hdr[:, 0:128]

        # all input DMAs issued up-front on the sync/SP HWDGE ring
        in_tiles = {}
        for i, (c0, cn) in enumerate(zip(g_starts, GROUPS)):
            ct = inp.tile([128, cn, 128], BF16, tag=f"in{i}", name=f"ct{i}")
            assert list(ct[:].ap[1])[0] == 128, "padded tile breaks _flat"
            nc.sync.dma_start(
                out=_flat(ct[:], cn * 128),
                in_=xb_d[:, HDR + c0 * 128:HDR + (c0 + cn) * 128])
            for c in range(c0, c0 + cn):
                in_tiles[c] = (ct, c - c0)

        # edge-column matmul: psE[:, i, e] = Lq @ (4*x_col{0,127}) = the
        # final out cols {0,1}/{126,127} values for image i, in f32.
        psE = ppE.tile([128, C_PER_CORE, 2], F32)
        nc.tensor.matmul(psE[:], lt, hdr[:, 128:256], start=True, stop=True)

        pending_store = None

        def issue_store(g):
            c0, cn = g_starts[g], GROUPS[g]
            ot, _ = out_tiles[g]
            nc.scalar.dma_start(
                out=o_d[:, c0 * 128:(c0 + cn) * 128],
                in_=_flat(ot[:], cn * 128))

        out_tiles = {}
        for g, (i0, G) in enumerate(zip(g_starts, GROUPS)):
            ct, cofs = in_tiles[i0]

            # pairsum-in-PSUM: P = Lq@evens + Lq@odds, two accumulating
            # matmuls per 512-col PSUM bank (8 images each).
            pt = pp.tile([128, 16, 64], F32, tag="P")
            for h in range(G // 8):
                po = pt[:, 8 * h:8 * h + 8, :]
                im = cofs + 8 * h
                nc.tensor.matmul(po, lt, ct[:, im:im + 8, 0:64],
                                 start=True, stop=False)
                nc.tensor.matmul(po, lt, ct[:, im:im + 8, 64:128],
                                 start=False, stop=True)

            # evacuate P to SBUF bf16 (ACT, one big contiguous copy)
            ps = mp.tile([128, 16, 64], BF16, tag="P16")
            assert list(ps[:].ap[1])[0] == 64 and list(pt[:].ap[1])[0] == 64
            nc.scalar.copy(out=ps[:, 0:G, :], in_=pt[:, 0:G, :])

            ot = iop.tile([128, G, 128], BF16, tag=f"out{g}", name=f"ot{g}")
            assert list(ot[:].ap[1])[0] == 128, "padded tile breaks _flat"
            out_tiles[g] = (ot, 0)
            pd = list(ps[:].ap[0])
            gdim = [64, G]
            od0 = list(ot[:].ap[0])
            ogdim = [128, G]

            # interior fold: out[4a+b] = P[2a+b-2] + P[2a+b], a=1..30,
            # split by b-parity so every read/write is a 4B-aligned
            # 2-element bf16 run (DVE 2x_1P eligible).
            nc.vector.tensor_add(
                out=_as_strided(ot[:, 0:G, 4:5], [od0, ogdim, [4, 30], [1, 2]]),
                in0=_as_strided(ps[:, 0:1, 0:1], [pd, gdim, [2, 30], [1, 2]]),
                in1=_as_strided(ps[:, 0:1, 2:3], [pd, gdim, [2, 30], [1, 2]]))
            nc.vector.tensor_add(
                out=_as_strided(ot[:, 0:G, 6:7], [od0, ogdim, [4, 30], [1, 2]]),
                in0=_as_strided(ps[:, 0:1, 2:3], [pd, gdim, [2, 30], [1, 2]]),
                in1=_as_strided(ps[:, 0:1, 4:5], [pd, gdim, [2, 30], [1, 2]]))

            # e1: out cols {0,1,126,127} <- psumE duplicated (DVE copy)
            pe = psE[:, i0:i0 + G, 0:1]
            nc.vector.tensor_copy(
                out=_as_strided(ot[:, 0:G, 0:1], [od0, ogdim, [126, 2], [1, 2]]),
                in_=_as_strided(pe, [list(pe.ap[0]), [2, G], [1, 2], [0, 2]]))

            # e3: out cols {2,3,124,125} = 2 * P slots {0,1,62,63} (GpSimd)
            nc.gpsimd.tensor_scalar_mul(
                _as_strided(ot[:, 0:G, 2:3], [od0, ogdim, [122, 2], [1, 2]]),
                _as_strided(ps[:, 0:1, 0:1], [pd, gdim, [62, 2], [1, 2]]),
                2.0)

            # stores ride the ACT ring, issued one group late so their
            # cross-engine waits are already satisfied when ACT gets there
            if pending_store is not None:
                issue_store(pending_store)
            pending_store = g
        issue_store(pending_store)


_CACHE = {}


def _get_nc():
    if "nc" not in _CACHE:
        nc = bacc.Bacc("TRN2", target_bir_lowering=False, debug=False)
        xb_d = nc.dram_tensor("xb", (128, HDR + C_PER_CORE * 128), BF16,
                              kind="ExternalInput").ap()
        o_d = nc.dram_tensor("o", (128, C_PER_CORE * 128), BF16,
                             kind="ExternalOutput").ap()
        with tile.TileContext(nc) as tc:
            _dpconv_tile(tc, o_d, xb_d)
        nc.compile()
        _CACHE["nc"] = nc
    return _CACHE["nc"]


def _stage(xk: np.ndarray) -> np.ndarray:
    """[C,H,W] f32 -> [128, HDR + C*128] bf16: header [lt | 4*edge-cols],
    then per image [evens | odds], H-major so DMA reads are one long
    contiguous run per partition."""
    t = xk.transpose(1, 0, 2)                      # [H, C, W]
    out = np.empty((128, HDR + C_PER_CORE * 128), np.float32)
    out[:, 0:128] = _LQ_T
    out[:, 128:256:2] = 4.0 * t[:, :, 0]
    out[:, 129:256:2] = 4.0 * t[:, :, 127]
    img = np.concatenate([t[:, :, 0::2], t[:, :, 1::2]], axis=2)
    out[:, 256:] = img.reshape(128, -1)
    return out.astype(BF16_NP)


def run(x: np.ndarray, **spmd_kwargs) -> bass_utils.BassKernelResults:
    """Shard x (8,64,128,128) across 8 cores and run the Bass kernel."""
    nc = _get_nc()
    in_maps = [{"xb": _stage(x[k])} for k in range(N_CORES)]
    return bass_utils.run_bass_kernel_spmd(
        nc, in_maps, core_ids=list(range(N_CORES)), **spmd_kwargs)


def kernel(x) -> np.ndarray:
    x = np.asarray(x, dtype=np.float32)
    assert x.shape == (N_CORES, C_PER_CORE, 128, 128), x.shape
    res = run(x)
    return np.stack(
        [res.results[k]["o"].reshape(128, C_PER_CORE, 128)
         .astype(np.float32).transpose(1, 0, 2)
         for k in range(N_CORES)],
        axis=0)


# revision 5
# speedup vs baseline: 1.1818x; 1.0723x over previous
"""DPConv (kernel=8, ext=4, stride=4) on 8 TRN2 NeuronCores — v9.

Math: with K = k + 2e = 16 and k = 8, every adaptive-pool bin is exactly
2 wide, so DPConv collapses to out = L @ img @ L.T per (n, c) image,
where L (128x128, entries n/16) is the exact 1-D operator.  The column
pass factors through natural pairs P[s] = x[2s] + x[2s+1]:

    out[:, 4a+b] = P[:, 2a+b-2] + P[:, 2a+b]      (a = 1..30, b = 0..3)
    out[:, {0,1}]   = T(x_col0)        out[:, {2,3}]     = 2 P[:, {0,1}]
    out[:, {126,127}] = T(x_col127)    out[:, {124,125}] = 2 P[:, {62,63}]

(P carries the matmul's Lq = L/4 scaling, which absorbs the 1/4.)

On-chip structure — the pairsum rides the matmul for free: the host
stages each image as [evens(64) | odds(64)] and the kernel issues TWO
accumulating matmuls per PSUM region (start/stop flags):
PSUM <- Lq@evens + Lq@odds = P directly.  A header block [lt | E]
(E = [4*x_col0, 4*x_col127] per image) rides in load chunk 0; one extra
matmul produces psumE = the final outer edge column values.

Engine assignment (v9) — each engine has ONE role so no instruction
queues behind a load-dependent wait:
  Sync:  load DMA issues (SP HWDGE ring) — 4 chunks, chunk0 = hdr+16img
  PE:    matmuls
  ACT:   P evacuation PSUM->SBUF bf16, e1/e3 edge columns
  DVE:   fold_even + fold_odd (4B-aligned 2-element runs -> 2x_1P mode)
  GpSimd: store DMA issues via SWDGE (qPoolDynamic) — the critical v9
         fix: v8 put stores on ACT, where Tile's in-order queue parked
         them behind the next group's evacuation, which waits on that
         group's matmuls and load — stores started only after ALL loads
         finished.  On the SWDGE queue a store waits only on its own
         producers, so store packets interleave with load packets in
         the shared SDMA engines from ~mid-kernel on.
Total DMA count = 8 = the 8 DMAHW completion lanes, so no store ever
waits to reuse a load's DMA semaphore.

Sharding: pure data parallel — core k takes batch element n = k.
"""

import ml_dtypes
import numpy as np

import concourse.bacc as bacc
import concourse.mybir as mybir
import concourse.tile as tile
from concourse import bass_utils
from concourse.ap import AP

N_CORES = 8
C_PER_CORE = 64          # images per core (= C; one batch element per core)
G = 16                   # images per compute group / load chunk / store chunk
N_GROUPS = C_PER_CORE // G
HDR = 256                # header cols: lt(128) + E(128), rides in chunk 0
F32 = mybir.dt.float32
BF16 = mybir.dt.bfloat16
BF16_NP = ml_dtypes.bfloat16


def _build_lq() -> np.ndarray:
    """The 1-D DPConv operator with the column-pass 1/4 folded in: L/4."""
    L = np.zeros((128, 128), np.float64)
    for w in range(128):
        i_lo = max(0, -((7 - w) // 4))      # ceil((w-7)/4)
        i_hi = min(30, w // 4)
        for i in (i_lo, i_hi):              # counted twice when equal
            L[w, min(127, max(0, 2 * w - 4 * i - 4))] += 0.25
            L[w, min(127, max(0, 2 * w - 4 * i - 3))] += 0.25
    return (L / 4.0).astype(np.float32)


_LQ_T = np.ascontiguousarray(_build_lq().T)          # lhsT layout [r, h]
_LQ_T_BF16 = _LQ_T.astype(BF16_NP)
assert np.all(_LQ_T_BF16.astype(np.float32) == _LQ_T)  # L exact in bf16


def _as_strided(base: AP, dims) -> AP:
    """Rebuild `base` (a sliced AP pointing at the wanted offset) with
    explicit [stride, size] free dims (overlapping reads allowed)."""
    return AP(base.tensor, base.offset, dims)


def _flat(ap: AP, n: int) -> AP:
    """Collapse a multi-dim access pattern into [128, n] so the DMA
    builder emits one long descriptor per partition."""
    pdim = list(ap.ap[0])
    return AP(ap.tensor, ap.offset, [pdim, [1, n]])


def _dpconv_tile(tc, o_d, xb_d):
    nc = tc.nc
    with tc.tile_pool(name="const", bufs=1) as cp, \
         tc.tile_pool(name="in", bufs=1) as inp, \
         tc.tile_pool(name="io", bufs=1) as iop, \
         tc.tile_pool(name="mid", bufs=3) as mp, \
         tc.tile_pool(name="ps", bufs=3, space="PSUM") as pp, \
         tc.tile_pool(name="psE", bufs=1, space="PSUM") as ppE:
        # chunk 0 = header + first 16 images in ONE load: the header costs
        # no extra DMA issue and lt/E are in SBUF the moment c0 lands.
        c0t = cp.tile([128, HDR + G * 128], BF16)
        nc.sync.dma_start(out=c0t[:], in_=xb_d[:, 0:HDR + G * 128])
        lt = c0t[:, 0:128]

        img_tiles = {0: (c0t, HDR)}          # group -> (tile, col offset)
        for i in range(1, N_GROUPS):
            ct = inp.tile([128, G, 128], BF16, tag=f"in{i}", name=f"ct{i}")
            assert list(ct[:].ap[1])[0] == 128, "padded tile breaks _flat"
            nc.sync.dma_start(
                out=_flat(ct[:], G * 128),
                in_=xb_d[:, HDR + i * G * 128:HDR + (i + 1) * G * 128])
            img_tiles[i] = (ct, 0)

        # edge-column matmul: psE[:, i, e] = Lq @ (4*x_col{0,127}) = the
        # final out cols {0,1}/{126,127} values for image i, in f32.
        psE = ppE.tile([128, C_PER_CORE, 2], F32)
        nc.tensor.matmul(psE[:], lt, c0t[:, 128:256], start=True, stop=True)

        for g in range(N_GROUPS):
            ct, cofs = img_tiles[g]

            def img_ap(i, lo, hi):
                """rhs AP over 8 staged images: cols [lo,hi) of each."""
                off = ct[:].offset + cofs + i * 128 + lo
                return AP(ct[:].tensor, off,
                          [list(ct[:].ap[0]), [128, 8], [1, hi - lo]])

            # pairsum-in-PSUM: P = Lq@evens + Lq@odds, two accumulating
            # matmuls per 512-col PSUM bank (8 images each).
            pt = pp.tile([128, G, 64], F32, tag="P")
            assert list(pt[:].ap[1])[0] == 64
            for h in range(G // 8):
                po = pt[:, 8 * h:8 * h + 8, :]
                nc.tensor.matmul(po, lt, img_ap(8 * h, 0, 64),
                                 start=True, stop=False)
                nc.tensor.matmul(po, lt, img_ap(8 * h, 64, 128),
                                 start=False, stop=True)

            # evacuate P to SBUF bf16 (ACT, one big contiguous copy)
            ps = mp.tile([128, G, 64], BF16, tag="P16")
            assert list(ps[:].ap[1])[0] == 64
            nc.scalar.copy(out=ps[:], in_=pt[:])

            ot = iop.tile([128, G, 128], BF16, tag=f"out{g}", name=f"ot{g}")
            assert list(ot[:].ap[1])[0] == 128, "padded tile breaks _flat"
            pd = list(ps[:].ap[0])
            gdim = [64, G]
            od0 = list(ot[:].ap[0])
            ogdim = [128, G]

            # interior fold on DVE: out[4a+b] = P[2a+b-2] + P[2a+b],
            # a=1..30, split by b-parity so every read/write is a
            # 4B-aligned 2-element bf16 run (2x_1P eligible).
            nc.vector.tensor_add(
                out=_as_strided(ot[:, 0:1, 4:5], [od0, ogdim, [4, 30], [1, 2]]),
                in0=_as_strided(ps[:, 0:1, 0:1], [pd, gdim, [2, 30], [1, 2]]),
                in1=_as_strided(ps[:, 0:1, 2:3], [pd, gdim, [2, 30], [1, 2]]))
            nc.vector.tensor_add(
                out=_as_strided(ot[:, 0:1, 6:7], [od0, ogdim, [4, 30], [1, 2]]),
                in0=_as_strided(ps[:, 0:1, 2:3], [pd, gdim, [2, 30], [1, 2]]),
                in1=_as_strided(ps[:, 0:1, 4:5], [pd, gdim, [2, 30], [1, 2]]))

            # e1 (ACT): out cols {0,1,126,127} <- psumE duplicated
            pe = psE[:, g * G:g * G + G, 0:1]
            nc.scalar.copy(
                out=_as_strided(ot[:, 0:1, 0:1], [od0, ogdim, [126, 2], [1, 2]]),
                in_=_as_strided(pe, [list(pe.ap[0]), [2, G], [1, 2], [0, 2]]))
            # e3 (ACT): out cols {2,3,124,125} = 2 * P slots {0,1,62,63}
            nc.scalar.mul(
                _as_strided(ot[:, 0:1, 2:3], [od0, ogdim, [122, 2], [1, 2]]),
                _as_strided(ps[:, 0:1, 0:1], [pd, gdim, [62, 2], [1, 2]]),
                2.0)

            # store via SWDGE (GpSimd queue): waits only on this group's
            # fold/edge producers, so store packets interleave with the
            # remaining load packets in the shared SDMA engines.
            nc.gpsimd.dma_start(
                out=o_d[:, g * G * 128:(g + 1) * G * 128],
                in_=_flat(ot[:], G * 128))


_CACHE = {}


def _get_nc():
    if "nc" not in _CACHE:
        nc = bacc.Bacc("TRN2", target_bir_lowering=False, debug=False)
        xb_d = nc.dram_tensor("xb", (128, HDR + C_PER_CORE * 128), BF16,
                              kind="ExternalInput").ap()
        o_d = nc.dram_tensor("o", (128, C_PER_CORE * 128), BF16,
                             kind="ExternalOutput").ap()
        with tile.TileContext(nc) as tc:
            _dpconv_tile(tc, o_d, xb_d)
        nc.compile()
        _CACHE["nc"] = nc
    return _CACHE["nc"]


def _stage(xk: np.ndarray) -> np.ndarray:
    """[C,H,W] f32 -> [128, HDR + C*128] bf16: header [lt | 4*edge-cols],
    then per image [evens | odds], H-major so DMA reads are one long
    contiguous run per partition."""
    t = xk.transpose(1, 0, 2)                      # [H, C, W]
    out = np.empty((128, HDR + C_PER_CORE * 128), np.float32)
    out[:, 0:128] = _LQ_T
    out[:, 128:256:2] = 4.0 * t[:, :, 0]
    out[:, 129:256:2] = 4.0 * t[:, :, 127]
    img = np.concatenate([t[:, :, 0::2], t[:, :, 1::2]], axis=2)
    out[:, 256:] = img.reshape(128, -1)
    return out.astype(BF16_NP)


def run(x: np.ndarray, **spmd_kwargs) -> bass_utils.BassKernelResults:
    """Shard x (8,64,128,128) across 8 cores and run the Bass kernel."""
    nc = _get_nc()
    in_maps = [{"xb": _stage(x[k])} for k in range(N_CORES)]
    return bass_utils.run_bass_kernel_spmd(
        nc, in_maps, core_ids=list(range(N_CORES)), **spmd_kwargs)


def kernel(x) -> np.ndarray:
    x = np.asarray(x, dtype=np.float32)
    assert x.shape == (N_CORES, C_PER_CORE, 128, 128), x.shape
    res = run(x)
    return np.stack(
        [res.results[k]["o"].reshape(128, C_PER_CORE, 128)
         .astype(np.float32).transpose(1, 0, 2)
         for k in range(N_CORES)],
        axis=0)
